# revision 28
# baseline (speedup 1.0000x reference)
"""Trainium2 Bass kernel v4 for nn_LocalAggregation (ball-query KNN + MLP + max).

Math: out[c, m] = relu( max_{k in NN32(m)} Gt[k, c] - Ht[m, c] ) with
Gt = (diag(s)W) @ [fea; xyz/R] per key, Ht per query, and ball-query masking
replacing far neighbors with self (slot 0).

v4 = v2 selection core + spatial pruning:
  Points z-sorted per batch. Keys laid out chunk-interleaved: column
  (c, s) holds z-rank 16*s + c, so each 512-key chunk is a uniform
  spatial sample (keeps top-8-per-chunk valid) while positions within a
  chunk are z-ordered. Each 128-query tile (z-slab) only scans key
  positions whose z lies within [z_tile_min - R, z_tile_max + R]: a
  static per-tile column window [LO_t, HI_t) shared by all chunks
  (~2.5x fewer keys scanned). Exact: any in-ball key is in-window.
  Cores take alternating 128-query stripes (core h owns global tiles
  2t+h) so one SPMD program serves all 8 cores with shared windows.

  Packing: w = (bits(dist+2^-6) & ~8191) ^ (0x7FFFFFFF ^ z_rank)
  13-bit GLOBAL rank in the low bits -> Max8 per chunk, 4x Max8 + 3x
  MatchReplace merge, one-op unpack (no per-chunk index recovery).
  Idx wrap for SWDGE: 8 one-hot PE matmuls replicate/transpose idx to the
  wrapped [16-partition, 8x-replicated] layout; ACT interleaves; GPSIMD
  casts. ONE dma_gather (4096 idxs) per tile on rotating SWDGE queues.

Sharding: 8 cores = 4 batch x 2 stripe-halves (4096 queries x 8192 keys).
"""

import numpy as np

import concourse.bacc as bacc
import concourse.mybir as mybir
from concourse import tile
from concourse.bass_utils import run_bass_kernel_spmd

B, C, N = 4, 64, 8192
K = 32
RADIUS = 0.2
R2 = RADIUS * RADIUS
EPS = 1e-5
CIN = C + 3
NCORES = 8
QPC = N // 2
NT = QPC // 128        # 32 query tiles per core
NCH = N // 512         # 16 key chunks
CHW = 512              # positions per chunk
RANK = 25              # bf16-split rank
OFF = 2.0 ** -6        # distance offset to keep exponents in a safe band
XOR_C = 0x7FFFFFFF

f32 = mybir.dt.float32
bf16 = mybir.dt.bfloat16
i32 = mybir.dt.int32
i16 = mybir.dt.int16
u16 = mybir.dt.uint16
u32 = mybir.dt.uint32

_CACHE = {}
_BOUNDS = None         # per-tile (LO, HI) column windows, set by _make_in_maps

# packed-domain threshold: masked <=> dist > R2 <=> w < wthr  (12-bit idx)
_THRESH_BITS = int(np.float32(R2 + OFF).view(np.int32))
WTHR = (_THRESH_BITS & ~4095) ^ XOR_C


def _build(reps=1, bounds=None):
    assert bounds is not None and len(bounds) == NT
    nc = bacc.Bacc("TRN2", target_bir_lowering=False, debug=False,
                   num_devices=NCORES, num_swdge_queues=4)

    a17_in = nc.dram_tensor("a17", [RANK, QPC], bf16, kind="ExternalInput").ap()
    b17_in = nc.dram_tensor("b17", [RANK, N], bf16, kind="ExternalInput").ap()
    xorrow_in = nc.dram_tensor("xorrow", [128, N], i32, kind="ExternalInput").ap()
    repm_in = nc.dram_tensor("repm", [128, 8 * 128], f32, kind="ExternalInput").ap()
    fea_in = nc.dram_tensor("fea", [C, N], f32, kind="ExternalInput").ap()
    xyz_in = nc.dram_tensor("xyz", [3, N], f32, kind="ExternalInput").ap()
    xyzq_in = nc.dram_tensor("xyzq", [3, QPC], f32, kind="ExternalInput").ap()
    w_in = nc.dram_tensor("w", [C, CIN], f32, kind="ExternalInput").ap()
    bnt_in = nc.dram_tensor("bnt", [C, 4], f32, kind="ExternalInput").ap()
    ones_in = nc.dram_tensor("onesrow", [1, QPC], f32, kind="ExternalInput").ap()
    y_out = nc.dram_tensor("y", [QPC, C], f32, kind="ExternalOutput").ap()

    cc_dram = nc.dram_tensor("cc_scr", [C, 1], f32).ap()
    wct_dram = nc.dram_tensor("wct_scr", [3, C], f32).ap()
    gt_dram = nc.dram_tensor("gt", [N, C], f32).ap()

    with tile.TileContext(nc) as tc:
        with tc.tile_pool(name="persist", bufs=1) as pp:
            a17 = pp.tile([RANK, QPC], bf16, tag="a17")
            b17 = pp.tile([RANK, N], bf16, tag="b17")
            xorrow = pp.tile([128, N], i32, tag="xorrow")
            repm = pp.tile([128, 8 * 128], f32, tag="repm")
            a4 = pp.tile([4, QPC], f32, tag="a4")
            rhs4 = pp.tile([4, C], f32, tag="rhs4")
            nc.sync.dma_start(out=a17[:], in_=a17_in[:])
            nc.sync.dma_start(out=b17[:], in_=b17_in[:])
            nc.sync.dma_start(out=xorrow[:], in_=xorrow_in[:])
            nc.sync.dma_start(out=repm[:], in_=repm_in[:])
            nc.sync.dma_start(out=a4[0:3, :], in_=xyzq_in[:])
            nc.sync.dma_start(out=a4[3:4, :], in_=ones_in[:])

            # ---------------- prep: s, cc, W', Gt ----------------
            with tc.tile_pool(name="prep", bufs=1) as sp, \
                 tc.tile_pool(name="prep_ps", bufs=2, space="PSUM") as pps:
                f67 = sp.tile([CIN, N], f32)
                w = sp.tile([C, CIN], f32)
                bnt = sp.tile([C, 4], f32)
                nc.sync.dma_start(out=f67[:C, :], in_=fea_in[:])
                nc.sync.dma_start(out=f67[C:, :], in_=xyz_in[:])
                nc.sync.dma_start(out=w[:], in_=w_in[:])
                nc.sync.dma_start(out=bnt[:], in_=bnt_in[:])

                # s = gamma / sqrt(var + eps); cc = s*mean - beta
                s_t = sp.tile([C, 1], f32)
                tmp = sp.tile([C, 1], f32)
                nc.vector.tensor_scalar_add(tmp[:], bnt[:, 3:4], EPS)
                nc.scalar.activation(tmp[:], tmp[:],
                                     mybir.ActivationFunctionType.Sqrt)
                nc.vector.reciprocal(tmp[:], tmp[:])
                nc.vector.tensor_mul(s_t[:], bnt[:, 0:1], tmp[:])
                cc_t = sp.tile([C, 1], f32)
                nc.vector.tensor_mul(cc_t[:], bnt[:, 2:3], s_t[:])
                nc.vector.tensor_sub(cc_t[:], cc_t[:], bnt[:, 1:2])
                nc.sync.dma_start(out=cc_dram[:], in_=cc_t[:])

                # W' = diag(s) @ W ; coor columns * (1/R)
                wp = sp.tile([C, CIN], f32)
                nc.vector.tensor_scalar_mul(wp[:], w[:], s_t[:])
                nc.vector.tensor_scalar_mul(wp[:, C:], wp[:, C:], 1.0 / RADIUS)

                # W'T via PE transpose against identity
                diag = sp.tile([C, C], f32)
                nc.gpsimd.memset(diag[:], 0.0)
                one_col = sp.tile([C, 1], f32)
                nc.gpsimd.memset(one_col[:], 1.0)
                nc.gpsimd.affine_select(
                    diag[:], one_col[:].to_broadcast([C, C]),
                    pattern=[[-1, C]], base=0, channel_multiplier=1,
                    compare_op=mybir.AluOpType.is_equal, fill=0.0)
                wpt_ps = pps.tile([CIN, C], f32)
                nc.tensor.matmul(wpt_ps[:], wp[:], diag[:], start=True, stop=True)
                wpt = sp.tile([CIN, C], f32)
                nc.scalar.copy(wpt[:], wpt_ps[:])
                nc.sync.dma_start(out=wct_dram[:], in_=wpt[C:, :])

                # rhs4 = [W'T coor rows ; ccT]
                nc.sync.dma_start(out=rhs4[0:3, :], in_=wct_dram[:])
                nc.sync.dma_start(out=rhs4[3:4, :],
                                  in_=cc_dram[:].rearrange("c one -> one c"))

                # Gt[n, c] = sum_p F67[p, n] * W'T[p, c] -> DRAM [N, C]
                gstage = sp.tile([128, (N // 128) * C], f32)
                for blk in range(N // 128):
                    gps = pps.tile([128, C], f32, tag="gps")
                    nc.tensor.matmul(gps[:], f67[:, blk * 128:(blk + 1) * 128],
                                     wpt[:], start=True, stop=True)
                    nc.scalar.copy(gstage[:, blk * C:(blk + 1) * C], gps[:])
                nc.sync.dma_start(
                    out=gt_dram[:].rearrange("(blk p) c -> p blk c", p=128),
                    in_=gstage[:].rearrange("p (blk c) -> p blk c", c=C))

            # ---------------- main loop ----------------
            with tc.tile_pool(name="nd_ps", bufs=4, space="PSUM") as ndp, \
                 tc.tile_pool(name="ht_ps", bufs=2, space="PSUM") as htp, \
                 tc.tile_pool(name="idx_ps", bufs=2, space="PSUM") as ixps, \
                 tc.tile_pool(name="chunk", bufs=4) as cp, \
                 tc.tile_pool(name="small", bufs=2) as smp, \
                 tc.tile_pool(name="idxp", bufs=4) as ixp, \
                 tc.tile_pool(name="gath", bufs=5) as gp:
                for rep in range(reps):
                    LAG = 3
                    pend = {}
                    for tt in range(NT + LAG):
                      if tt >= LAG:
                        # drain tile tt-LAG: reduce + Ht + output
                        q0d, gathd = pend.pop(tt - LAG)
                        gmax = smp.tile([128, C], f32, tag="gmax")
                        nc.vector.reduce_max(
                            out=gmax[:],
                            in_=gathd[:].rearrange("p (s c) -> p c s", s=K),
                            axis=mybir.AxisListType.X)
                        hp = htp.tile([128, C], f32, tag="hps")
                        nc.tensor.matmul(hp[:], a4[:, q0d:q0d + 128], rhs4[:],
                                         start=True, stop=True)
                        ht = smp.tile([128, C], f32, tag="ht")
                        nc.scalar.copy(ht[:], hp[:])
                        o = smp.tile([128, C], f32, tag="o")
                        nc.vector.tensor_sub(o[:], gmax[:], ht[:])
                        nc.vector.tensor_scalar_max(o[:], o[:], 0.0)
                        nc.sync.dma_start(out=y_out[q0d:q0d + 128, :], in_=o[:])
                      if tt < NT:
                        t = tt
                        q0 = t * 128
                        lo, hi = bounds[t]
                        wdt = hi - lo
                        cand = smp.tile([128, NCH * 8], f32, tag="cand")
                        for c in range(NCH):
                            sl = slice(c * CHW + lo, c * CHW + hi)
                            ps = ndp.tile([128, CHW], f32, tag="nd")
                            nc.tensor.matmul(ps[:, :wdt], a17[:, q0:q0 + 128],
                                             b17[:, sl], start=True, stop=True)
                            dist = cp.tile([128, CHW], f32, tag="dist")
                            nc.scalar.copy(dist[:, :wdt], ps[:, :wdt])
                            wpk = cp.tile([128, CHW], i32, tag="wpk")
                            pk = nc.vector.scalar_tensor_tensor(
                                out=wpk[:, :wdt],
                                in0=dist[:, :wdt].bitcast(i32),
                                scalar=-4096, in1=xorrow[:, sl],
                                op0=mybir.AluOpType.bitwise_and,
                                op1=mybir.AluOpType.bitwise_xor)
                            pk.ins.ins[1].dtype = i32
                            nc.vector.max(out=cand[:, c * 8:c * 8 + 8],
                                          in_=wpk[:, :wdt].bitcast(f32))

                        # exact top-32 of the 128 packed candidates
                        wsel = smp.tile([128, K], f32, tag="wsel")
                        work = smp.tile([128, NCH * 8], f32, tag="work")
                        src = cand
                        for it in range(4):
                            nc.vector.max(out=wsel[:, it * 8:it * 8 + 8],
                                          in_=src[:])
                            if it < 3:
                                nc.vector.match_replace(
                                    out=work[:],
                                    in_to_replace=wsel[:, it * 8:it * 8 + 8],
                                    in_values=src[:], imm_value=0.0)
                                src = work

                        # unpack: rank mod 4096 = (w & 4095) ^ 4095, then
                        # rel = (rankm - 16*lo) & 4095  (window span < 4096)
                        rkm = smp.tile([128, K], i32, tag="rkm")
                        up = nc.vector.tensor_scalar(
                            rkm[:], wsel[:].bitcast(i32), 4095, 4095,
                            op0=mybir.AluOpType.bitwise_and,
                            op1=mybir.AluOpType.bitwise_xor)
                        up.ins.ins[1].dtype = i32
                        up.ins.ins[2].dtype = i32
                        rks = smp.tile([128, K], i32, tag="rks")
                        up2 = nc.vector.tensor_scalar(
                            rks[:], rkm[:], (16 * lo) & 4095, None,
                            op0=mybir.AluOpType.subtract)
                        up2.ins.ins[1].dtype = i32
                        idxs = smp.tile([128, K], i32, tag="idxs")
                        up3 = nc.vector.tensor_scalar(
                            idxs[:], rks[:], 4095, None,
                            op0=mybir.AluOpType.bitwise_and)
                        up3.ins.ins[1].dtype = i32
                        mask = smp.tile([128, K], u32, tag="mask")
                        mk = nc.vector.tensor_scalar(
                            mask[:], wsel[:].bitcast(i32), WTHR, None,
                            op0=mybir.AluOpType.is_lt)
                        mk.ins.ins[1].dtype = i32
                        nc.vector.copy_predicated(
                            idxs[:], mask[:], idxs[:, 0:1].to_broadcast([128, K]))
                        idxf = smp.tile([128, K], f32, tag="idxf")
                        nc.gpsimd.tensor_copy(out=idxf[:], in_=idxs[:])

                        # wrap for SWDGE: psum[p, g*32+s] = idxf[g*16+p%16, s]
                        # (8 one-hot matmuls; result replicated over p//16)
                        ixq = ixps.tile([128, 8 * K], f32, tag="ixq")
                        for g in range(8):
                            nc.tensor.matmul(
                                ixq[:, g * K:(g + 1) * K],
                                repm[:, g * 128:(g + 1) * 128], idxf[:],
                                start=True, stop=True)
                        idxw_f = ixp.tile([128, 8 * K], f32, tag="idxw_f")
                        nc.scalar.copy(
                            idxw_f[:].rearrange("p (s eight) -> p eight s",
                                                eight=8),
                            ixq[:].rearrange("p (eight s) -> p eight s", s=K))
                        idxw = ixp.tile([128, 8 * K], u16, tag="idxw")
                        nc.gpsimd.tensor_copy(out=idxw[:], in_=idxw_f[:])

                        gath = gp.tile([128, K * C], f32, tag="gath")
                        for gq in range(4):
                            nc.gpsimd.dma_gather(
                                out_ap=gath[:, gq * 8 * C:(gq + 1) * 8 * C
                                            ].rearrange("p (s c) -> p s c", s=8),
                                in_ap=gt_dram[16 * lo:, :],
                                idxs_ap=idxw[:, gq * 64:(gq + 1) * 64
                                             ].bitcast(i16),
                                num_idxs=8 * 128, num_idxs_reg=8 * 128,
                                elem_size=C, single_packet=True,
                                queue_num=gq)

                        pend[t] = (q0, gath)

    nc.compile()
    return nc


def _get_nc(reps=1):
    key = ("nc", reps, _BOUNDS)
    if key not in _CACHE:
        _CACHE[key] = _build(reps, bounds=_BOUNDS)
    return _CACHE[key]


def _bf16(x):
    b = np.ascontiguousarray(x, np.float32).view(np.uint32)
    r = ((b + 0x7FFF + ((b >> 16) & 1)) & 0xFFFF0000).astype(np.uint32)
    return r.view(np.float32)


def _split_rows(xq, xk, sqq, sqk):
    """rank-25 bf16 split rows for dist+OFF = sqq + sqk - 2 x.x + OFF."""
    ones_q = np.ones((1, xq.shape[1]), np.float32)
    ones_k = np.ones((1, xk.shape[1]), np.float32)
    xq_hi = _bf16(xq); xq_md = _bf16(xq - xq_hi)
    xq_lo = _bf16(xq - xq_hi - xq_md)
    xk_hi = _bf16(xk); xk_md = _bf16(xk - xk_hi)
    xk_lo = _bf16(xk - xk_hi - xk_md)
    sqq_hi = _bf16(sqq); sqq_md = _bf16(sqq - sqq_hi)
    sqq_lo = _bf16(sqq - sqq_hi - sqq_md)
    sqk_hi = _bf16(sqk); sqk_md = _bf16(sqk - sqk_hi)
    sqk_lo = _bf16(sqk - sqk_hi - sqk_md)
    a_rows, b_rows = [], []
    for c in range(3):
        a_rows += [xq_hi[c], xq_hi[c], xq_hi[c],
                   xq_md[c], xq_md[c], xq_lo[c]]
        b_rows += [-2 * xk_hi[c], -2 * xk_md[c], -2 * xk_lo[c],
                   -2 * xk_hi[c], -2 * xk_md[c], -2 * xk_hi[c]]
    a_rows += [sqq_hi[0], sqq_md[0], sqq_lo[0],
               ones_q[0], ones_q[0], ones_q[0], ones_q[0]]
    b_rows += [ones_k[0], ones_k[0], ones_k[0],
               sqk_hi[0], sqk_md[0], sqk_lo[0], OFF * ones_k[0]]
    import ml_dtypes
    a17 = _bf16(np.stack(a_rows)).astype(ml_dtypes.bfloat16)
    b17 = _bf16(np.stack(b_rows)).astype(ml_dtypes.bfloat16)
    return a17, b17


# column (c, s) of the chunk-interleaved layout holds z-rank 16*s + c
_COLMAP = (16 * (np.arange(N) % CHW) + np.arange(N) // CHW).astype(np.int64)


def _make_in_maps(inputs):
    global _BOUNDS
    points_coor = np.ascontiguousarray(inputs["points_coor"], np.float32)
    points_fea = np.ascontiguousarray(inputs["points_fea"], np.float32)
    W = np.ascontiguousarray(inputs["W"], np.float32)
    bnt = np.ascontiguousarray(
        np.stack([inputs["gamma"], inputs["beta"], inputs["running_mean"],
                  inputs["running_var"]], axis=1), np.float32)
    xorrow = np.tile((XOR_C ^ (_COLMAP & 4095)).astype(np.int32)[None, :],
                     (128, 1))
    # repm[p, g*128 + r*16 + w] = 1 iff p == g*16 + w  (one-hot wrap/replicate)
    repm = np.zeros((128, 8 * 128), np.float32)
    for g in range(8):
        for w_ in range(16):
            for r in range(8):
                repm[g * 16 + w_, g * 128 + r * 16 + w_] = 1.0
    onesrow = np.ones((1, QPC), np.float32)

    # z-sort per batch; per-global-tile key-rank windows
    orders, zss = [], []
    rlo = np.full(2 * NT, N, np.int64)
    rhi = np.zeros(2 * NT, np.int64)
    for b in range(B):
        order = np.argsort(points_coor[b, 2], kind="stable")
        orders.append(order)
        zs = points_coor[b, 2][order]
        zss.append(zs)
        for g in range(2 * NT):
            zqmin = zs[g * 128]
            zqmax = zs[g * 128 + 127]
            rlo[g] = min(rlo[g], np.searchsorted(zs, zqmin - RADIUS, "left"))
            rhi[g] = max(rhi[g], np.searchsorted(zs, zqmax + RADIUS, "right"))
    bounds = []
    for t in range(NT):
        lo_r = min(rlo[2 * t], rlo[2 * t + 1])
        hi_r = max(rhi[2 * t], rhi[2 * t + 1])
        lo = max(0, int(lo_r - 15) // 16)
        hi = min(CHW, int(hi_r) // 16 + 1)
        bounds.append((lo, hi))
    _BOUNDS = tuple(bounds)

    in_maps = []
    for core in range(NCORES):
        b, h = core // 2, core % 2
        order = orders[b]
        xyz_s = points_coor[b][:, order]          # rank-ordered keys
        fea_s = points_fea[b][:, order]
        # stripe queries: core h owns global tiles 2t+h
        qsel = np.concatenate(
            [np.arange(128 * (2 * t + h), 128 * (2 * t + h) + 128)
             for t in range(NT)])
        xyzq = np.ascontiguousarray(xyz_s[:, qsel])
        sqk_r = (xyz_s * xyz_s).sum(0, keepdims=True)
        sqq = np.ascontiguousarray(sqk_r[:, qsel])
        # chunk-interleaved key layout for b17
        xk_cp = np.ascontiguousarray(xyz_s[:, _COLMAP])
        sqk_cp = np.ascontiguousarray(sqk_r[:, _COLMAP])
        a17, b17 = _split_rows(xyzq, xk_cp, sqq, sqk_cp)
        in_maps.append(dict(
            a17=a17, b17=b17, xorrow=xorrow, repm=repm,
            fea=fea_s, xyz=xyz_s, xyzq=xyzq,
            w=W, bnt=bnt, onesrow=onesrow,
        ))
    return in_maps


def kernel(points_coor, points_fea, W, gamma, beta, running_mean, running_var,
           **_unused):
    inputs = dict(points_coor=points_coor, points_fea=points_fea, W=W,
                  gamma=gamma, beta=beta, running_mean=running_mean,
                  running_var=running_var)
    in_maps = _make_in_maps(inputs)
    nc = _get_nc()
    res = run_bass_kernel_spmd(nc, in_maps, list(range(NCORES)))
    out = np.empty((B, C, N), np.float32)
    points_coor = np.ascontiguousarray(inputs["points_coor"], np.float32)
    for core in range(NCORES):
        b, h = core // 2, core % 2
        order = np.argsort(points_coor[b, 2], kind="stable")
        qsel = np.concatenate(
            [np.arange(128 * (2 * t + h), 128 * (2 * t + h) + 128)
             for t in range(NT)])
        out[b, :, order[qsel]] = res.results[core]["y"]
    return out


# revision 29
# speedup vs baseline: 1.0099x; 1.0099x over previous
"""Trainium2 Bass kernel v4 for nn_LocalAggregation (ball-query KNN + MLP + max).

Math: out[c, m] = relu( max_{k in NN32(m)} Gt[k, c] - Ht[m, c] ) with
Gt = (diag(s)W) @ [fea; xyz/R] per key, Ht per query, and ball-query masking
replacing far neighbors with self (slot 0).

v4 = v2 selection core + spatial pruning:
  Points z-sorted per batch. Keys laid out chunk-interleaved: column
  (c, s) holds z-rank 16*s + c, so each 512-key chunk is a uniform
  spatial sample (keeps top-8-per-chunk valid) while positions within a
  chunk are z-ordered. Each 128-query tile (z-slab) only scans key
  positions whose z lies within [z_tile_min - R, z_tile_max + R]: a
  static per-tile column window [LO_t, HI_t) shared by all chunks
  (~2.5x fewer keys scanned). Exact: any in-ball key is in-window.
  Cores take alternating 128-query stripes (core h owns global tiles
  2t+h) so one SPMD program serves all 8 cores with shared windows.

  Packing: w = (bits(dist+2^-6) & ~8191) ^ (0x7FFFFFFF ^ z_rank)
  13-bit GLOBAL rank in the low bits -> Max8 per chunk, 4x Max8 + 3x
  MatchReplace merge, one-op unpack (no per-chunk index recovery).
  Idx wrap for SWDGE: 8 one-hot PE matmuls replicate/transpose idx to the
  wrapped [16-partition, 8x-replicated] layout; ACT interleaves; GPSIMD
  casts. ONE dma_gather (4096 idxs) per tile on rotating SWDGE queues.

Sharding: 8 cores = 4 batch x 2 stripe-halves (4096 queries x 8192 keys).
"""

import numpy as np

import concourse.bacc as bacc
import concourse.mybir as mybir
from concourse import tile
from concourse.bass_utils import run_bass_kernel_spmd

B, C, N = 4, 64, 8192
K = 32
RADIUS = 0.2
R2 = RADIUS * RADIUS
EPS = 1e-5
CIN = C + 3
NCORES = 8
QPC = N // 2
NT = QPC // 128        # 32 query tiles per core
NCH = N // 512         # 16 key chunks
CHW = 512              # positions per chunk
RANK = 25              # bf16-split rank
OFF = 2.0 ** -6        # distance offset to keep exponents in a safe band
XOR_C = 0x7FFFFFFF

f32 = mybir.dt.float32
bf16 = mybir.dt.bfloat16
i32 = mybir.dt.int32
i16 = mybir.dt.int16
u16 = mybir.dt.uint16
u32 = mybir.dt.uint32

_CACHE = {}
_BOUNDS = None         # per-tile (LO, HI) column windows, set by _make_in_maps

# packed-domain threshold: masked <=> dist > R2 <=> w < wthr  (12-bit idx)
_THRESH_BITS = int(np.float32(R2 + OFF).view(np.int32))
WTHR = (_THRESH_BITS & ~4095) ^ XOR_C


def _build(reps=1, bounds=None):
    assert bounds is not None and len(bounds) == NT
    nc = bacc.Bacc("TRN2", target_bir_lowering=False, debug=False,
                   num_devices=NCORES, num_swdge_queues=4,
                   dynamic_dma_scratch_size=65536)

    a17_in = nc.dram_tensor("a17", [RANK, QPC], bf16, kind="ExternalInput").ap()
    b17_in = nc.dram_tensor("b17", [RANK, N], bf16, kind="ExternalInput").ap()
    xorrow_in = nc.dram_tensor("xorrow", [128, N], i32, kind="ExternalInput").ap()
    repm_in = nc.dram_tensor("repm", [128, 8 * 128], f32, kind="ExternalInput").ap()
    fea_in = nc.dram_tensor("fea", [C, N], f32, kind="ExternalInput").ap()
    xyz_in = nc.dram_tensor("xyz", [3, N], f32, kind="ExternalInput").ap()
    xyzq_in = nc.dram_tensor("xyzq", [3, QPC], f32, kind="ExternalInput").ap()
    w_in = nc.dram_tensor("w", [C, CIN], f32, kind="ExternalInput").ap()
    bnt_in = nc.dram_tensor("bnt", [C, 4], f32, kind="ExternalInput").ap()
    ones_in = nc.dram_tensor("onesrow", [1, QPC], f32, kind="ExternalInput").ap()
    y_out = nc.dram_tensor("y", [QPC, C], f32, kind="ExternalOutput").ap()

    cc_dram = nc.dram_tensor("cc_scr", [C, 1], f32).ap()
    wct_dram = nc.dram_tensor("wct_scr", [3, C], f32).ap()
    gt_dram = nc.dram_tensor("gt", [N, C], f32).ap()

    with tile.TileContext(nc) as tc:
        with tc.tile_pool(name="persist", bufs=1) as pp:
            a17 = pp.tile([RANK, QPC], bf16, tag="a17")
            b17 = pp.tile([RANK, N], bf16, tag="b17")
            xorrow = pp.tile([128, N], i32, tag="xorrow")
            repm = pp.tile([128, 8 * 128], f32, tag="repm")
            a4 = pp.tile([4, QPC], f32, tag="a4")
            rhs4 = pp.tile([4, C], f32, tag="rhs4")
            nc.sync.dma_start(out=a17[:], in_=a17_in[:])
            nc.sync.dma_start(out=b17[:], in_=b17_in[:])
            nc.sync.dma_start(out=xorrow[:], in_=xorrow_in[:])
            nc.sync.dma_start(out=repm[:], in_=repm_in[:])
            nc.sync.dma_start(out=a4[0:3, :], in_=xyzq_in[:])
            nc.sync.dma_start(out=a4[3:4, :], in_=ones_in[:])

            # ---------------- prep: s, cc, W', Gt ----------------
            with tc.tile_pool(name="prep", bufs=1) as sp, \
                 tc.tile_pool(name="prep_ps", bufs=2, space="PSUM") as pps:
                f67 = sp.tile([CIN, N], f32)
                w = sp.tile([C, CIN], f32)
                bnt = sp.tile([C, 4], f32)
                nc.sync.dma_start(out=f67[:C, :], in_=fea_in[:])
                nc.sync.dma_start(out=f67[C:, :], in_=xyz_in[:])
                nc.sync.dma_start(out=w[:], in_=w_in[:])
                nc.sync.dma_start(out=bnt[:], in_=bnt_in[:])

                # s = gamma / sqrt(var + eps); cc = s*mean - beta
                s_t = sp.tile([C, 1], f32)
                tmp = sp.tile([C, 1], f32)
                nc.vector.tensor_scalar_add(tmp[:], bnt[:, 3:4], EPS)
                nc.scalar.activation(tmp[:], tmp[:],
                                     mybir.ActivationFunctionType.Sqrt)
                nc.vector.reciprocal(tmp[:], tmp[:])
                nc.vector.tensor_mul(s_t[:], bnt[:, 0:1], tmp[:])
                cc_t = sp.tile([C, 1], f32)
                nc.vector.tensor_mul(cc_t[:], bnt[:, 2:3], s_t[:])
                nc.vector.tensor_sub(cc_t[:], cc_t[:], bnt[:, 1:2])
                nc.sync.dma_start(out=cc_dram[:], in_=cc_t[:])

                # W' = diag(s) @ W ; coor columns * (1/R)
                wp = sp.tile([C, CIN], f32)
                nc.vector.tensor_scalar_mul(wp[:], w[:], s_t[:])
                nc.vector.tensor_scalar_mul(wp[:, C:], wp[:, C:], 1.0 / RADIUS)

                # W'T via PE transpose against identity
                diag = sp.tile([C, C], f32)
                nc.gpsimd.memset(diag[:], 0.0)
                one_col = sp.tile([C, 1], f32)
                nc.gpsimd.memset(one_col[:], 1.0)
                nc.gpsimd.affine_select(
                    diag[:], one_col[:].to_broadcast([C, C]),
                    pattern=[[-1, C]], base=0, channel_multiplier=1,
                    compare_op=mybir.AluOpType.is_equal, fill=0.0)
                wpt_ps = pps.tile([CIN, C], f32)
                nc.tensor.matmul(wpt_ps[:], wp[:], diag[:], start=True, stop=True)
                wpt = sp.tile([CIN, C], f32)
                nc.scalar.copy(wpt[:], wpt_ps[:])
                nc.sync.dma_start(out=wct_dram[:], in_=wpt[C:, :])

                # rhs4 = [W'T coor rows ; ccT]
                nc.sync.dma_start(out=rhs4[0:3, :], in_=wct_dram[:])
                nc.sync.dma_start(out=rhs4[3:4, :],
                                  in_=cc_dram[:].rearrange("c one -> one c"))

                # Gt[n, c] = sum_p F67[p, n] * W'T[p, c] -> DRAM [N, C]
                gstage = sp.tile([128, (N // 128) * C], f32)
                for blk in range(N // 128):
                    gps = pps.tile([128, C], f32, tag="gps")
                    nc.tensor.matmul(gps[:], f67[:, blk * 128:(blk + 1) * 128],
                                     wpt[:], start=True, stop=True)
                    nc.scalar.copy(gstage[:, blk * C:(blk + 1) * C], gps[:])
                nc.sync.dma_start(
                    out=gt_dram[:].rearrange("(blk p) c -> p blk c", p=128),
                    in_=gstage[:].rearrange("p (blk c) -> p blk c", c=C))

            # ---------------- main loop ----------------
            with tc.tile_pool(name="nd_ps", bufs=4, space="PSUM") as ndp, \
                 tc.tile_pool(name="ht_ps", bufs=2, space="PSUM") as htp, \
                 tc.tile_pool(name="idx_ps", bufs=2, space="PSUM") as ixps, \
                 tc.tile_pool(name="chunk", bufs=4) as cp, \
                 tc.tile_pool(name="small", bufs=2) as smp, \
                 tc.tile_pool(name="idxp", bufs=4) as ixp, \
                 tc.tile_pool(name="gath", bufs=5) as gp:
                for rep in range(reps):
                    LAG = 3
                    pend = {}
                    for tt in range(NT + LAG):
                      if tt >= LAG:
                        # drain tile tt-LAG: reduce + Ht + output
                        q0d, gathd = pend.pop(tt - LAG)
                        gmax = smp.tile([128, C], f32, tag="gmax")
                        nc.vector.reduce_max(
                            out=gmax[:],
                            in_=gathd[:].rearrange("p (s c) -> p c s", s=K),
                            axis=mybir.AxisListType.X)
                        hp = htp.tile([128, C], f32, tag="hps")
                        nc.tensor.matmul(hp[:], a4[:, q0d:q0d + 128], rhs4[:],
                                         start=True, stop=True)
                        ht = smp.tile([128, C], f32, tag="ht")
                        nc.scalar.copy(ht[:], hp[:])
                        o = smp.tile([128, C], f32, tag="o")
                        nc.vector.tensor_sub(o[:], gmax[:], ht[:])
                        nc.vector.tensor_scalar_max(o[:], o[:], 0.0)
                        nc.sync.dma_start(out=y_out[q0d:q0d + 128, :], in_=o[:])
                      if tt < NT:
                        t = tt
                        q0 = t * 128
                        lo, hi = bounds[t]
                        wdt = hi - lo
                        cand = smp.tile([128, NCH * 8], f32, tag="cand")
                        for c in range(NCH):
                            sl = slice(c * CHW + lo, c * CHW + hi)
                            ps = ndp.tile([128, CHW], f32, tag="nd")
                            nc.tensor.matmul(ps[:, :wdt], a17[:, q0:q0 + 128],
                                             b17[:, sl], start=True, stop=True)
                            dist = cp.tile([128, CHW], f32, tag="dist")
                            nc.scalar.copy(dist[:, :wdt], ps[:, :wdt])
                            wpk = cp.tile([128, CHW], i32, tag="wpk")
                            pk = nc.vector.scalar_tensor_tensor(
                                out=wpk[:, :wdt],
                                in0=dist[:, :wdt].bitcast(i32),
                                scalar=-4096, in1=xorrow[:, sl],
                                op0=mybir.AluOpType.bitwise_and,
                                op1=mybir.AluOpType.bitwise_xor)
                            pk.ins.ins[1].dtype = i32
                            nc.vector.max(out=cand[:, c * 8:c * 8 + 8],
                                          in_=wpk[:, :wdt].bitcast(f32))

                        # exact top-32 of the 128 packed candidates
                        wsel = smp.tile([128, K], f32, tag="wsel")
                        work = smp.tile([128, NCH * 8], f32, tag="work")
                        src = cand
                        for it in range(4):
                            nc.vector.max(out=wsel[:, it * 8:it * 8 + 8],
                                          in_=src[:])
                            if it < 3:
                                nc.vector.match_replace(
                                    out=work[:],
                                    in_to_replace=wsel[:, it * 8:it * 8 + 8],
                                    in_values=src[:], imm_value=0.0)
                                src = work

                        # unpack: rank mod 4096 = (w & 4095) ^ 4095, then
                        # rel = (rankm - 16*lo) & 4095  (window span < 4096)
                        rkm = smp.tile([128, K], i32, tag="rkm")
                        up = nc.vector.tensor_scalar(
                            rkm[:], wsel[:].bitcast(i32), 4095, 4095,
                            op0=mybir.AluOpType.bitwise_and,
                            op1=mybir.AluOpType.bitwise_xor)
                        up.ins.ins[1].dtype = i32
                        up.ins.ins[2].dtype = i32
                        rks = smp.tile([128, K], i32, tag="rks")
                        up2 = nc.vector.tensor_scalar(
                            rks[:], rkm[:], (16 * lo) & 4095, None,
                            op0=mybir.AluOpType.subtract)
                        up2.ins.ins[1].dtype = i32
                        idxs = smp.tile([128, K], i32, tag="idxs")
                        up3 = nc.vector.tensor_scalar(
                            idxs[:], rks[:], 4095, None,
                            op0=mybir.AluOpType.bitwise_and)
                        up3.ins.ins[1].dtype = i32
                        mask = smp.tile([128, K], u32, tag="mask")
                        mk = nc.vector.tensor_scalar(
                            mask[:], wsel[:].bitcast(i32), WTHR, None,
                            op0=mybir.AluOpType.is_lt)
                        mk.ins.ins[1].dtype = i32
                        nc.vector.copy_predicated(
                            idxs[:], mask[:], idxs[:, 0:1].to_broadcast([128, K]))
                        idxf = smp.tile([128, K], f32, tag="idxf")
                        nc.gpsimd.tensor_copy(out=idxf[:], in_=idxs[:])

                        # wrap for SWDGE: psum[p, g*32+s] = idxf[g*16+p%16, s]
                        # (8 one-hot matmuls; result replicated over p//16)
                        ixq = ixps.tile([128, 8 * K], f32, tag="ixq")
                        for g in range(8):
                            nc.tensor.matmul(
                                ixq[:, g * K:(g + 1) * K],
                                repm[:, g * 128:(g + 1) * 128], idxf[:],
                                start=True, stop=True)
                        idxw_f = ixp.tile([128, 8 * K], f32, tag="idxw_f")
                        nc.scalar.copy(
                            idxw_f[:].rearrange("p (s eight) -> p eight s",
                                                eight=8),
                            ixq[:].rearrange("p (eight s) -> p eight s", s=K))
                        idxw = ixp.tile([128, 8 * K], u16, tag="idxw")
                        nc.gpsimd.tensor_copy(out=idxw[:], in_=idxw_f[:])

                        gath = gp.tile([128, K * C], f32, tag="gath")
                        for gq in range(4):
                            nc.gpsimd.dma_gather(
                                out_ap=gath[:, gq * 8 * C:(gq + 1) * 8 * C
                                            ].rearrange("p (s c) -> p s c", s=8),
                                in_ap=gt_dram[16 * lo:, :],
                                idxs_ap=idxw[:, gq * 64:(gq + 1) * 64
                                             ].bitcast(i16),
                                num_idxs=8 * 128, num_idxs_reg=8 * 128,
                                elem_size=C, single_packet=True,
                                queue_num=gq)

                        pend[t] = (q0, gath)

    nc.compile()
    return nc


def _get_nc(reps=1):
    key = ("nc", reps, _BOUNDS)
    if key not in _CACHE:
        _CACHE[key] = _build(reps, bounds=_BOUNDS)
    return _CACHE[key]


def _bf16(x):
    b = np.ascontiguousarray(x, np.float32).view(np.uint32)
    r = ((b + 0x7FFF + ((b >> 16) & 1)) & 0xFFFF0000).astype(np.uint32)
    return r.view(np.float32)


def _split_rows(xq, xk, sqq, sqk):
    """rank-25 bf16 split rows for dist+OFF = sqq + sqk - 2 x.x + OFF."""
    ones_q = np.ones((1, xq.shape[1]), np.float32)
    ones_k = np.ones((1, xk.shape[1]), np.float32)
    xq_hi = _bf16(xq); xq_md = _bf16(xq - xq_hi)
    xq_lo = _bf16(xq - xq_hi - xq_md)
    xk_hi = _bf16(xk); xk_md = _bf16(xk - xk_hi)
    xk_lo = _bf16(xk - xk_hi - xk_md)
    sqq_hi = _bf16(sqq); sqq_md = _bf16(sqq - sqq_hi)
    sqq_lo = _bf16(sqq - sqq_hi - sqq_md)
    sqk_hi = _bf16(sqk); sqk_md = _bf16(sqk - sqk_hi)
    sqk_lo = _bf16(sqk - sqk_hi - sqk_md)
    a_rows, b_rows = [], []
    for c in range(3):
        a_rows += [xq_hi[c], xq_hi[c], xq_hi[c],
                   xq_md[c], xq_md[c], xq_lo[c]]
        b_rows += [-2 * xk_hi[c], -2 * xk_md[c], -2 * xk_lo[c],
                   -2 * xk_hi[c], -2 * xk_md[c], -2 * xk_hi[c]]
    a_rows += [sqq_hi[0], sqq_md[0], sqq_lo[0],
               ones_q[0], ones_q[0], ones_q[0], ones_q[0]]
    b_rows += [ones_k[0], ones_k[0], ones_k[0],
               sqk_hi[0], sqk_md[0], sqk_lo[0], OFF * ones_k[0]]
    import ml_dtypes
    a17 = _bf16(np.stack(a_rows)).astype(ml_dtypes.bfloat16)
    b17 = _bf16(np.stack(b_rows)).astype(ml_dtypes.bfloat16)
    return a17, b17


# column (c, s) of the chunk-interleaved layout holds z-rank 16*s + c
_COLMAP = (16 * (np.arange(N) % CHW) + np.arange(N) // CHW).astype(np.int64)


def _make_in_maps(inputs):
    global _BOUNDS
    points_coor = np.ascontiguousarray(inputs["points_coor"], np.float32)
    points_fea = np.ascontiguousarray(inputs["points_fea"], np.float32)
    W = np.ascontiguousarray(inputs["W"], np.float32)
    bnt = np.ascontiguousarray(
        np.stack([inputs["gamma"], inputs["beta"], inputs["running_mean"],
                  inputs["running_var"]], axis=1), np.float32)
    xorrow = np.tile((XOR_C ^ (_COLMAP & 4095)).astype(np.int32)[None, :],
                     (128, 1))
    # repm[p, g*128 + r*16 + w] = 1 iff p == g*16 + w  (one-hot wrap/replicate)
    repm = np.zeros((128, 8 * 128), np.float32)
    for g in range(8):
        for w_ in range(16):
            for r in range(8):
                repm[g * 16 + w_, g * 128 + r * 16 + w_] = 1.0
    onesrow = np.ones((1, QPC), np.float32)

    # z-sort per batch; per-global-tile key-rank windows
    orders, zss = [], []
    rlo = np.full(2 * NT, N, np.int64)
    rhi = np.zeros(2 * NT, np.int64)
    for b in range(B):
        order = np.argsort(points_coor[b, 2], kind="stable")
        orders.append(order)
        zs = points_coor[b, 2][order]
        zss.append(zs)
        for g in range(2 * NT):
            zqmin = zs[g * 128]
            zqmax = zs[g * 128 + 127]
            rlo[g] = min(rlo[g], np.searchsorted(zs, zqmin - RADIUS, "left"))
            rhi[g] = max(rhi[g], np.searchsorted(zs, zqmax + RADIUS, "right"))
    bounds = []
    for t in range(NT):
        lo_r = min(rlo[2 * t], rlo[2 * t + 1])
        hi_r = max(rhi[2 * t], rhi[2 * t + 1])
        lo = max(0, int(lo_r - 15) // 16)
        hi = min(CHW, int(hi_r) // 16 + 1)
        bounds.append((lo, hi))
    _BOUNDS = tuple(bounds)

    in_maps = []
    for core in range(NCORES):
        b, h = core // 2, core % 2
        order = orders[b]
        xyz_s = points_coor[b][:, order]          # rank-ordered keys
        fea_s = points_fea[b][:, order]
        # stripe queries: core h owns global tiles 2t+h
        qsel = np.concatenate(
            [np.arange(128 * (2 * t + h), 128 * (2 * t + h) + 128)
             for t in range(NT)])
        xyzq = np.ascontiguousarray(xyz_s[:, qsel])
        sqk_r = (xyz_s * xyz_s).sum(0, keepdims=True)
        sqq = np.ascontiguousarray(sqk_r[:, qsel])
        # chunk-interleaved key layout for b17
        xk_cp = np.ascontiguousarray(xyz_s[:, _COLMAP])
        sqk_cp = np.ascontiguousarray(sqk_r[:, _COLMAP])
        a17, b17 = _split_rows(xyzq, xk_cp, sqq, sqk_cp)
        in_maps.append(dict(
            a17=a17, b17=b17, xorrow=xorrow, repm=repm,
            fea=fea_s, xyz=xyz_s, xyzq=xyzq,
            w=W, bnt=bnt, onesrow=onesrow,
        ))
    return in_maps


def kernel(points_coor, points_fea, W, gamma, beta, running_mean, running_var,
           **_unused):
    inputs = dict(points_coor=points_coor, points_fea=points_fea, W=W,
                  gamma=gamma, beta=beta, running_mean=running_mean,
                  running_var=running_var)
    in_maps = _make_in_maps(inputs)
    nc = _get_nc()
    res = run_bass_kernel_spmd(nc, in_maps, list(range(NCORES)))
    out = np.empty((B, C, N), np.float32)
    points_coor = np.ascontiguousarray(inputs["points_coor"], np.float32)
    for core in range(NCORES):
        b, h = core // 2, core % 2
        order = np.argsort(points_coor[b, 2], kind="stable")
        qsel = np.concatenate(
            [np.arange(128 * (2 * t + h), 128 * (2 * t + h) + 128)
             for t in range(NT)])
        out[b, :, order[qsel]] = res.results[core]["y"]
    return out
